# revision 3
# baseline (speedup 1.0000x reference)
"""LoRALinear kernel for Trainium2 (8 NeuronCores, SPMD data-parallel).

Computes out = x @ W.T + b + SCALE*((x@gA.T)@gB.T + (x@lA.T)@lB.T)
  x: [8, 2048, 1024] f32, W: [4096, 1024], b: [4096]
  gA/lA: [8, 1024], gB/lB: [4096, 8]  ->  out: [8, 2048, 4096] f32

Strategy: the rank-16 LoRA update is folded into the weights on the
host (O(r*d_in*d_out) = 0.05% of total FLOPs) and both GEMM operands
are marshaled to the layout the PE array needs (contraction dim on
partitions), fp16:
  W_effT[k, o] = (W + SCALE*(gB@gA + lB@lA)).T   -> [1024, 4096] fp16
  xT[k, s]     = x[i].T per core                 -> [1024, 2048] fp16

Device (per core, one batch of x): a pure dense GEMM at the fp16
roofline (216 ns per 128x128x512 matmul at 2.4 GHz).  Both operands
stay resident in SBUF (96 KiB/partition).  The s-range is processed
in 4 quarters of 4 s-tiles; within a quarter, o-tiles are the outer
loop so the first psum groups need only ~1.25 MB of operands --
DMA chunks are issued in exactly the order compute consumes them so
the PE never starves after the lead-in.  Dummy K=1 matmuls plus the
bias-broadcast matmuls keep the PE busy during the DMA lead-in so the
HAM clock gate is warm (2.4 GHz) when the main stream begins.  Bias is
added in f32 on DVE during psum eviction.

fp16 operand rounding gives ~3e-4 absmax relative error vs the f32
reference; accumulation stays f32 in PSUM.
"""
import numpy as np
from contextlib import ExitStack

import concourse.bass as bass
import concourse.tile as tile
from concourse import bacc, mybir
from concourse.bass import ts, ds
from concourse.bass_utils import run_bass_kernel_spmd

F32 = mybir.dt.float32
F16 = mybir.dt.float16

N_CORES = 8
B, S, DIN, DOUT, R = 8, 2048, 1024, 4096, 8
SCALE = 16.0 / 8

P = 128            # partition tile
OTILE = 512        # matmul moving free dim (one PSUM bank of f32)
KT = DIN // P      # 8 k-tiles
OT = DOUT // OTILE # 8 o-tiles
ST = S // P        # 16 s-tiles
SQ = 4             # s-quarters (4 s-tiles each)
STQ = ST // SQ
NWARM = 6          # dummy PE warm-up matmuls during DMA lead-in


def build_nc():
    nc = bacc.Bacc("TRN2", target_bir_lowering=False, debug=False,
                   num_devices=N_CORES)
    xT = nc.dram_tensor("xT", [DIN, S], F16, kind="ExternalInput").ap()
    WeT = nc.dram_tensor("WeT", [DIN, DOUT], F16, kind="ExternalInput").ap()
    bvec = nc.dram_tensor("b16", [DOUT], F16, kind="ExternalInput").ap()
    out = nc.dram_tensor("out", [S, DOUT], F32, kind="ExternalOutput").ap()

    with tile.TileContext(nc) as tc:
        with ExitStack() as ctx:
            const = ctx.enter_context(tc.tile_pool(name="const", bufs=1))
            xw_pool = ctx.enter_context(tc.tile_pool(name="xw", bufs=1))
            out_pool = ctx.enter_context(tc.tile_pool(name="outp", bufs=4))
            psw = ctx.enter_context(tc.tile_pool(name="psw", bufs=1, space="PSUM"))
            ps512 = ctx.enter_context(tc.tile_pool(name="ps512", bufs=6, space="PSUM"))

            # ---- bias row DMA first (tiny; unblocks bias broadcast) ----
            brow16 = const.tile([1, DOUT], F16)
            nc.sync.dma_start(brow16[:], bvec[None, :])

            # ---- resident operands: x.T and W_eff.T, fp16 ----
            xsb = [xw_pool.tile([P, S], F16, name=f"x{k}") for k in range(KT)]
            wet = [xw_pool.tile([P, DOUT], F16, name=f"w{k}") for k in range(KT)]

            # DMA issue order == compute consumption order.
            # Lead-in: x cols 0:512 (s-quarter 0) + W cols 0:512 (o-tile 0).
            for k in range(KT):
                nc.sync.dma_start(xsb[k][:, 0:OTILE], xT[ds(k * P, P), 0:OTILE])
            for k in range(KT):
                nc.sync.dma_start(wet[k][:, 0:OTILE], WeT[ds(k * P, P), 0:OTILE])
            # W bulk: 1024-col chunks, in o-consumption order.
            for c0, c1 in ((512, 1536), (1536, 2560)):
                for k in range(KT):
                    nc.sync.dma_start(wet[k][:, c0:c1], WeT[ds(k * P, P), c0:c1])
            # x s-quarter 1, then rest of W, then x s-quarters 2-3.
            for k in range(KT):
                nc.sync.dma_start(xsb[k][:, 512:1024], xT[ds(k * P, P), 512:1024])
            for c0, c1 in ((2560, 3584), (3584, 4096)):
                for k in range(KT):
                    nc.sync.dma_start(wet[k][:, c0:c1], WeT[ds(k * P, P), c0:c1])
            for k in range(KT):
                nc.sync.dma_start(xsb[k][:, 1024:2048], xT[ds(k * P, P), 1024:2048])

            # ---- PE warm-up: keep HAM at 2.4 GHz during the DMA lead-in ----
            ones_col = const.tile([1, P], F16)
            nc.vector.memset(ones_col[:], 1.0)
            ones_row = const.tile([1, OTILE], F16)
            nc.vector.memset(ones_row[:], 1.0)
            warm = psw.tile([P, OTILE], F32)
            for _ in range(NWARM):
                nc.tensor.matmul(warm[:], ones_col[:], ones_row[:],
                                 start=True, stop=True)

            # ---- bias broadcast via rank-1 matmuls (tail of the warm-up) ----
            bias_sb = const.tile([P, DOUT], F32)
            for ot in range(OT):
                pb = ps512.tile([P, OTILE], F32, tag="ps512")
                nc.tensor.matmul(pb[:], ones_col[:],
                                 brow16[:, ts(ot, OTILE)],
                                 start=True, stop=True)
                nc.vector.tensor_copy(bias_sb[:, ts(ot, OTILE)], pb[:])

            # ---- main GEMM: out[s, o] = x @ W_effT + bias ----
            for sq in range(SQ):
                for ot in range(OT):
                    for stq in range(STQ):
                        st = sq * STQ + stq
                        po = ps512.tile([P, OTILE], F32, tag="ps512")
                        for kt in range(KT):
                            nc.tensor.matmul(po[:], xsb[kt][:, ts(st, P)],
                                             wet[kt][:, ts(ot, OTILE)],
                                             start=(kt == 0), stop=(kt == KT - 1))
                        osb = out_pool.tile([P, OTILE], F32)
                        nc.vector.tensor_tensor(osb[:], po[:],
                                                bias_sb[:, ts(ot, OTILE)],
                                                mybir.AluOpType.add)
                        nc.sync.dma_start(out[ts(st, P), ts(ot, OTILE)], osb[:])

    nc.compile()
    return nc


_NC_CACHE = None


def _get_nc():
    global _NC_CACHE
    if _NC_CACHE is None:
        _NC_CACHE = build_nc()
    return _NC_CACHE


def make_in_maps(x, W, b, global_A, global_B, local_A, local_B):
    x = np.asarray(x, dtype=np.float32)
    W = np.asarray(W, dtype=np.float32)
    b16 = np.asarray(b, dtype=np.float32).astype(np.float16)
    lora = (np.asarray(global_B, dtype=np.float32) @ np.asarray(global_A, dtype=np.float32)
            + np.asarray(local_B, dtype=np.float32) @ np.asarray(local_A, dtype=np.float32))
    WeT = np.ascontiguousarray((W + SCALE * lora).T).astype(np.float16)
    x16 = x.astype(np.float16)
    return [
        {"xT": np.ascontiguousarray(x16[i].T), "WeT": WeT, "b16": b16}
        for i in range(N_CORES)
    ]


def kernel(x, W, b, global_A, global_B, local_A, local_B):
    nc = _get_nc()
    in_maps = make_in_maps(x, W, b, global_A, global_B, local_A, local_B)
    res = run_bass_kernel_spmd(nc, in_maps, list(range(N_CORES))).results
    return np.stack([res[i]["out"] for i in range(N_CORES)], axis=0)


# revision 4
# speedup vs baseline: 1.1025x; 1.1025x over previous
"""LoRALinear kernel for Trainium2 (8 NeuronCores, SPMD data-parallel).

Computes out = x @ W.T + b + SCALE*((x@gA.T)@gB.T + (x@lA.T)@lB.T)
  x: [8, 2048, 1024] f32, W: [4096, 1024], b: [4096]
  gA/lA: [8, 1024], gB/lB: [4096, 8]  ->  out: [8, 2048, 4096] f32

Strategy: the rank-16 LoRA update is folded into the weights on the
host (O(r*d_in*d_out) = 0.05% of total FLOPs) and both GEMM operands
are marshaled to the [partition, k-tile, col] layout the PE array
needs (contraction dim on partitions), fp16:
  WeT3[p, k, o] = W_eff.T[k*128+p, o],  W_eff = W + SCALE*(gB@gA+lB@lA)
  xT3[p, k, s]  = x[i].T[k*128+p, s]   per core

Device (per core, one batch of x): a pure dense GEMM at the fp16
roofline (216 ns per 128x128x512 matmul at 2.4 GHz).  Both operands
stay resident in SBUF (96 KiB/partition).  TRN2 has two HWDGE rings,
each FIFO with ~0.6us fixed cost per dma_start: input chunks go on the
sync ring as single 1 MB 3D DMAs in exactly compute-consumption order
(W o-chunk j is in SBUF before pass j needs it); all 128 output stores
go on the scalar ring so they never queue behind inputs.  The s-range
is processed in 4 quarters; within a quarter o-tiles are outer so the
first psum group needs only 2 MB of operands.  Dummy K=1 matmuls plus
the bias-broadcast matmuls keep the PE busy during the DMA lead-in so
the HAM clock gate is warm (2.4 GHz) when the main stream begins.
Bias is added in f32 on DVE during psum eviction.

fp16 operand rounding gives ~3e-4 absmax relative error vs the f32
reference; accumulation stays f32 in PSUM.
"""
import numpy as np
from contextlib import ExitStack

import concourse.bass as bass
import concourse.tile as tile
from concourse import bacc, mybir
from concourse.bass import ts, ds
from concourse.bass_utils import run_bass_kernel_spmd

F32 = mybir.dt.float32
F16 = mybir.dt.float16

N_CORES = 8
B, S, DIN, DOUT, R = 8, 2048, 1024, 4096, 8
SCALE = 16.0 / 8

P = 128            # partition tile
OTILE = 512        # matmul moving free dim (one PSUM bank of f32)
KT = DIN // P      # 8 k-tiles
OT = DOUT // OTILE # 8 o-tiles
ST = S // P        # 16 s-tiles
SQ = 4             # s-quarters (4 s-tiles each)
STQ = ST // SQ
SQW = STQ * P      # columns of x per s-quarter


def build_nc():
    nc = bacc.Bacc("TRN2", target_bir_lowering=False, debug=False,
                   num_devices=N_CORES)
    xT3 = nc.dram_tensor("xT3", [P, KT, S], F16, kind="ExternalInput").ap()
    WeT3 = nc.dram_tensor("WeT3", [P, KT, DOUT], F16, kind="ExternalInput").ap()
    bvec = nc.dram_tensor("b16", [DOUT], F16, kind="ExternalInput").ap()
    out = nc.dram_tensor("out", [S, DOUT], F32, kind="ExternalOutput").ap()

    with tile.TileContext(nc) as tc:
        with ExitStack() as ctx:
            const = ctx.enter_context(tc.tile_pool(name="const", bufs=1))
            xw_pool = ctx.enter_context(tc.tile_pool(name="xw", bufs=1))
            out_pool = ctx.enter_context(tc.tile_pool(name="outp", bufs=4))
            psw = ctx.enter_context(tc.tile_pool(name="psw", bufs=1, space="PSUM"))
            ps512 = ctx.enter_context(tc.tile_pool(name="ps512", bufs=6, space="PSUM"))

            # ---- bias row DMA first (tiny; unblocks bias broadcast) ----
            brow16 = const.tile([1, DOUT], F16)
            nc.sync.dma_start(brow16[:], bvec[None, :])

            # ---- resident operands: x.T and W_eff.T, fp16, 3D tiles ----
            xsb = xw_pool.tile([P, KT, S], F16, name="xsb")
            wet = xw_pool.tile([P, KT, DOUT], F16, name="wet")

            # Input DMAs on the sync ring, one 1 MB DMA per chunk, issued in
            # exactly the order compute consumes them.
            nc.sync.dma_start(wet[:, :, 0:OTILE], WeT3[:, :, 0:OTILE])
            nc.sync.dma_start(xsb[:, :, 0:SQW], xT3[:, :, 0:SQW])
            for j in range(1, OT):
                nc.sync.dma_start(wet[:, :, ts(j, OTILE)], WeT3[:, :, ts(j, OTILE)])
            for q in range(1, SQ):
                nc.sync.dma_start(xsb[:, :, ts(q, SQW)], xT3[:, :, ts(q, SQW)])

            # ---- PE warm-up: keep HAM at 2.4 GHz during the DMA lead-in ----
            ones_col = const.tile([1, P], F16)
            nc.vector.memset(ones_col[:], 1.0)
            ones_row = const.tile([1, OTILE], F16)
            nc.vector.memset(ones_row[:], 1.0)
            warm = psw.tile([P, OTILE], F32)
            for _ in range(6):
                nc.tensor.matmul(warm[:], ones_col[:], ones_row[:],
                                 start=True, stop=True)

            # ---- bias broadcast via rank-1 matmuls (part of the warm-up) ----
            bias_sb = const.tile([P, DOUT], F32)
            for ot in range(OT):
                pb = ps512.tile([P, OTILE], F32, tag="ps512")
                nc.tensor.matmul(pb[:], ones_col[:],
                                 brow16[:, ts(ot, OTILE)],
                                 start=True, stop=True)
                nc.vector.tensor_copy(bias_sb[:, ts(ot, OTILE)], pb[:])
            for _ in range(6):
                nc.tensor.matmul(warm[:], ones_col[:], ones_row[:],
                                 start=True, stop=True)

            # ---- main GEMM: out[s, o] = x @ W_effT + bias ----
            for sq in range(SQ):
                for ot in range(OT):
                    for stq in range(STQ):
                        st = sq * STQ + stq
                        po = ps512.tile([P, OTILE], F32, tag="ps512")
                        for kt in range(KT):
                            nc.tensor.matmul(po[:], xsb[:, kt, ts(st, P)],
                                             wet[:, kt, ts(ot, OTILE)],
                                             start=(kt == 0), stop=(kt == KT - 1))
                        osb = out_pool.tile([P, OTILE], F32)
                        nc.vector.tensor_tensor(osb[:], po[:],
                                                bias_sb[:, ts(ot, OTILE)],
                                                mybir.AluOpType.add)
                        # output stores on the second HWDGE ring (scalar)
                        nc.scalar.dma_start(out[ts(st, P), ts(ot, OTILE)], osb[:])

    nc.compile()
    return nc


_NC_CACHE = None


def _get_nc():
    global _NC_CACHE
    if _NC_CACHE is None:
        _NC_CACHE = build_nc()
    return _NC_CACHE


def make_in_maps(x, W, b, global_A, global_B, local_A, local_B):
    x = np.asarray(x, dtype=np.float32)
    W = np.asarray(W, dtype=np.float32)
    b16 = np.asarray(b, dtype=np.float32).astype(np.float16)
    lora = (np.asarray(global_B, dtype=np.float32) @ np.asarray(global_A, dtype=np.float32)
            + np.asarray(local_B, dtype=np.float32) @ np.asarray(local_A, dtype=np.float32))
    W_eff16 = (W + SCALE * lora).astype(np.float16)        # [DOUT, DIN]
    # WeT3[p, k, o] = W_eff[o, k*128+p]
    WeT3 = np.ascontiguousarray(
        W_eff16.reshape(DOUT, KT, P).transpose(2, 1, 0))
    x16 = x.astype(np.float16)                             # [B, S, DIN]
    return [
        # xT3[p, k, s] = x[i][s, k*128+p]
        {"xT3": np.ascontiguousarray(x16[i].reshape(S, KT, P).transpose(2, 1, 0)),
         "WeT3": WeT3, "b16": b16}
        for i in range(N_CORES)
    ]


def kernel(x, W, b, global_A, global_B, local_A, local_B):
    nc = _get_nc()
    in_maps = make_in_maps(x, W, b, global_A, global_B, local_A, local_B)
    res = run_bass_kernel_spmd(nc, in_maps, list(range(N_CORES))).results
    return np.stack([res[i]["out"] for i in range(N_CORES)], axis=0)


# revision 6
# speedup vs baseline: 1.1111x; 1.0077x over previous
"""LoRALinear kernel for Trainium2 (8 NeuronCores, SPMD data-parallel).

Computes out = x @ W.T + b + SCALE*((x@gA.T)@gB.T + (x@lA.T)@lB.T)
  x: [8, 2048, 1024] f32, W: [4096, 1024], b: [4096]
  gA/lA: [8, 1024], gB/lB: [4096, 8]  ->  out: [8, 2048, 4096] f32

Strategy: the rank-16 LoRA update is folded into the weights on the
host (O(r*d_in*d_out) = 0.05% of total FLOPs) and both GEMM operands
are marshaled to the [partition, k-tile, col] layout the PE array
needs (contraction dim on partitions), fp16:
  WeT3[p, k, o] = W_eff.T[k*128+p, o],  W_eff = W + SCALE*(gB@gA+lB@lA)
  xT3[p, k, s]  = x[i].T[k*128+p, s]   per core

Device (per core, one batch of x): a pure dense GEMM at the fp16
roofline (216 ns per 128x128x512 matmul at 2.4 GHz).  Both operands
stay resident in SBUF (96 KiB/partition).  TRN2 has two HWDGE rings,
each FIFO with ~0.6us fixed cost per dma_start: input chunks go on the
sync ring as single 1 MB 3D DMAs in exactly compute-consumption order
(W o-chunk j is in SBUF before pass j needs it); all 128 output stores
go on the scalar ring so they never queue behind inputs.  The s-range
is processed in 4 quarters; within a quarter o-tiles are outer so the
first psum group needs only 2 MB of operands.  Dummy K=1 matmuls plus
the bias-broadcast matmuls keep the PE busy during the DMA lead-in so
the HAM clock gate is warm (2.4 GHz) when the main stream begins.
Bias is added in f32 on DVE during psum eviction.

fp16 operand rounding gives ~3e-4 absmax relative error vs the f32
reference; accumulation stays f32 in PSUM.
"""
import numpy as np
from contextlib import ExitStack

import concourse.bass as bass
import concourse.tile as tile
from concourse import bacc, mybir
from concourse.bass import ts, ds
from concourse.bass_utils import run_bass_kernel_spmd

F32 = mybir.dt.float32
F16 = mybir.dt.float16

N_CORES = 8
B, S, DIN, DOUT, R = 8, 2048, 1024, 4096, 8
SCALE = 16.0 / 8

P = 128            # partition tile
OTILE = 512        # matmul moving free dim (one PSUM bank of f32)
KT = DIN // P      # 8 k-tiles
OT = DOUT // OTILE # 8 o-tiles
ST = S // P        # 16 s-tiles
SQ = 4             # s-quarters (4 s-tiles each)
STQ = ST // SQ
SQW = STQ * P      # columns of x per s-quarter


def build_nc():
    nc = bacc.Bacc("TRN2", target_bir_lowering=False, debug=False,
                   num_devices=N_CORES)
    xT3 = nc.dram_tensor("xT3", [P, KT, S], F16, kind="ExternalInput").ap()
    WeT3 = nc.dram_tensor("WeT3", [P, KT, DOUT], F16, kind="ExternalInput").ap()
    bvec = nc.dram_tensor("b16", [DOUT], F16, kind="ExternalInput").ap()
    out = nc.dram_tensor("out", [S, DOUT], F32, kind="ExternalOutput").ap()

    with tile.TileContext(nc) as tc:
        with ExitStack() as ctx:
            const = ctx.enter_context(tc.tile_pool(name="const", bufs=1))
            xw_pool = ctx.enter_context(tc.tile_pool(name="xw", bufs=1))
            out_pool = ctx.enter_context(tc.tile_pool(name="outp", bufs=4))
            psw = ctx.enter_context(tc.tile_pool(name="psw", bufs=1, space="PSUM"))
            ps512 = ctx.enter_context(tc.tile_pool(name="ps512", bufs=6, space="PSUM"))

            # ---- bias row DMA first (tiny; unblocks bias broadcast) ----
            brow16 = const.tile([1, DOUT], F16)
            nc.sync.dma_start(brow16[:], bvec[None, :])

            # ---- resident operands: x.T and W_eff.T, fp16, 3D tiles ----
            xsb = xw_pool.tile([P, KT, S], F16, name="xsb")
            wet = xw_pool.tile([P, KT, DOUT], F16, name="wet")

            # Input DMAs on the sync ring, issued in exactly the order
            # compute consumes them; first s-tile of x split out so the
            # first psum group can start ASAP.
            nc.sync.dma_start(xsb[:, :, 0:P], xT3[:, :, 0:P])
            nc.sync.dma_start(wet[:, :, 0:OTILE], WeT3[:, :, 0:OTILE])
            nc.sync.dma_start(xsb[:, :, P:2 * P], xT3[:, :, P:2 * P])
            nc.sync.dma_start(xsb[:, :, 2 * P:SQW], xT3[:, :, 2 * P:SQW])
            for j in range(1, OT):
                nc.sync.dma_start(wet[:, :, ts(j, OTILE)], WeT3[:, :, ts(j, OTILE)])
            for q in range(1, SQ):
                nc.sync.dma_start(xsb[:, :, ts(q, SQW)], xT3[:, :, ts(q, SQW)])

            # ---- PE warm-up: full-K matmuls on memset data keep the whole
            # array busy during the DMA lead-in so the HAM clock gate is at
            # 2.4 GHz when the main stream begins (K=1 matmuls don't count
            # enough PE activity to un-throttle it).
            ones_col = const.tile([1, P], F16)
            nc.vector.memset(ones_col[:], 1.0)
            wsrc = const.tile([P, P + OTILE], F16)
            nc.vector.memset(wsrc[:], 1.0)
            warm = psw.tile([P, OTILE], F32)
            for _ in range(12):
                nc.tensor.matmul(warm[:], wsrc[:, 0:P], wsrc[:, P:P + OTILE],
                                 start=True, stop=True)

            # ---- bias broadcast via rank-1 matmuls (tail of the warm-up) ----
            bias_sb = const.tile([P, DOUT], F32)
            for ot in range(OT):
                pb = ps512.tile([P, OTILE], F32, tag="ps512")
                nc.tensor.matmul(pb[:], ones_col[:],
                                 brow16[:, ts(ot, OTILE)],
                                 start=True, stop=True)
                nc.vector.tensor_copy(bias_sb[:, ts(ot, OTILE)], pb[:])

            # ---- main GEMM: out[s, o] = x @ W_effT + bias ----
            for sq in range(SQ):
                for ot in range(OT):
                    for stq in range(STQ):
                        st = sq * STQ + stq
                        po = ps512.tile([P, OTILE], F32, tag="ps512")
                        for kt in range(KT):
                            nc.tensor.matmul(po[:], xsb[:, kt, ts(st, P)],
                                             wet[:, kt, ts(ot, OTILE)],
                                             start=(kt == 0), stop=(kt == KT - 1))
                        osb = out_pool.tile([P, OTILE], F32)
                        # output stores on the second HWDGE ring (scalar);
                        # the very last group is split 4-ways to pipeline
                        # the eviction/store latency off the critical tail
                        last = (st == ST - 1 and ot == OT - 1)
                        nsplit = 4 if last else 1
                        w = OTILE // nsplit
                        for c in range(nsplit):
                            nc.vector.tensor_tensor(
                                osb[:, ds(c * w, w)], po[:, ds(c * w, w)],
                                bias_sb[:, ds(ot * OTILE + c * w, w)],
                                mybir.AluOpType.add)
                            nc.scalar.dma_start(
                                out[ts(st, P), ds(ot * OTILE + c * w, w)],
                                osb[:, ds(c * w, w)])

    nc.compile()
    return nc


_NC_CACHE = None


def _get_nc():
    global _NC_CACHE
    if _NC_CACHE is None:
        _NC_CACHE = build_nc()
    return _NC_CACHE


def make_in_maps(x, W, b, global_A, global_B, local_A, local_B):
    x = np.asarray(x, dtype=np.float32)
    W = np.asarray(W, dtype=np.float32)
    b16 = np.asarray(b, dtype=np.float32).astype(np.float16)
    lora = (np.asarray(global_B, dtype=np.float32) @ np.asarray(global_A, dtype=np.float32)
            + np.asarray(local_B, dtype=np.float32) @ np.asarray(local_A, dtype=np.float32))
    W_eff16 = (W + SCALE * lora).astype(np.float16)        # [DOUT, DIN]
    # WeT3[p, k, o] = W_eff[o, k*128+p]
    WeT3 = np.ascontiguousarray(
        W_eff16.reshape(DOUT, KT, P).transpose(2, 1, 0))
    x16 = x.astype(np.float16)                             # [B, S, DIN]
    return [
        # xT3[p, k, s] = x[i][s, k*128+p]
        {"xT3": np.ascontiguousarray(x16[i].reshape(S, KT, P).transpose(2, 1, 0)),
         "WeT3": WeT3, "b16": b16}
        for i in range(N_CORES)
    ]


def kernel(x, W, b, global_A, global_B, local_A, local_B):
    nc = _get_nc()
    in_maps = make_in_maps(x, W, b, global_A, global_B, local_A, local_B)
    res = run_bass_kernel_spmd(nc, in_maps, list(range(N_CORES))).results
    return np.stack([res[i]["out"] for i in range(N_CORES)], axis=0)
